# revision 9
# baseline (speedup 1.0000x reference)
"""Trainium2 Bass kernel for nn_DecoderWithAttention (B=100 captioning decoder:
per-step additive attention over R=15 regions, two LSTM cells, two V=10000
heads, NT=5 steps).

Strategy: 8-way tensor parallelism with the (padded) batch replicated via
per-step AllGathers; data-parallel preamble. Each core owns
  - a 128-row slice of both LSTMs' hidden dim (gate rows i/f/g/o),
  - a 1280-row slice of each vocab head,
  - a 512-wide slice of the image-feature dim (for the attended embedding),
  - 13 of the 104 (padded) samples for the preamble (autoencoder topics,
    attention keys att1, image mean).
Activations are feature-major (features on SBUF partitions, batch on the free
axis) so AllGather's partition-axis concat lands shards directly in matmul
layout. Weights are bf16 (fp32 PSUM accumulation); biases fold into ScalarE
activation evictions. The image-mean and last-word-embedding contributions to
LSTM1's gates are precomputed once (they are step-constant / step-indexed).
"""

import sys

sys.path.insert(0, "/opt/trn_rl_repo")

import numpy as np
import ml_dtypes

bf16 = ml_dtypes.bfloat16

B, R, F = 100, 15, 4096
V, E, D, A, T = 10000, 1024, 1024, 512, 500
NT, L = 5, 52
NC = 8
BP = 104   # padded batch = 8 * 13
BS = 13    # per-core preamble batch shard
VP = 1280  # per-core vocab rows (padded vocab 10240)
FS = F // NC  # 512, per-core feature slice for awe

_STATE = {}


def _build_program():
    import concourse.mybir as mybir
    import concourse.tile as tile
    from concourse import bacc
    from concourse.ap import AP

    dt = mybir.dt
    AF = mybir.ActivationFunctionType
    OP = mybir.AluOpType
    X = mybir.AxisListType.X

    nc = bacc.Bacc("TRN2", target_bir_lowering=False, debug=False, num_devices=NC)

    def din(name, shape, dtype):
        return nc.dram_tensor(name, list(shape), dtype, kind="ExternalInput")

    bf = dt.bfloat16
    f32 = dt.float32

    # ---- external inputs (host-prepped per-core shards) ----
    img_dp = din("img_dp", (F, BS, R), bf)
    img_f = din("img_f", (FS, BP, R), bf)
    w_feat = din("w_feat", (F, A), bf)
    w_lt = din("w_lt", (F, 1024), bf)
    blt = din("blt", (128, 1024), f32)
    cwt = din("cwt", (120, 4, 6), bf)
    cbt = din("cbt", (128, 6), f32)
    w1h2 = din("w1h2", (1024, 512), bf)
    w1img = din("w1img", (F, 512), bf)
    w1emb = din("w1emb", (1024, 512), bf)
    whh1 = din("whh1", (1024, 512), bf)
    b1 = din("b1", (128, 4), f32)
    w2awe = din("w2awe", (F, 512), bf)
    w2h1 = din("w2h1", (1024, 512), bf)
    whh2 = din("whh2", (1024, 512), bf)
    b2 = din("b2", (128, 4), f32)
    wdec = din("wdec", (1024, 512), bf)
    wtop = din("wtop", (512, 512), bf)
    battn = din("battn", (128, 4), f32)
    wfull = din("wfull", (128, 4), bf)
    wfc1 = din("wfc1", (1024, VP), bf)
    bfc1 = din("bfc1", (128, 10), f32)
    wfc = din("wfc", (1024, VP), bf)
    bfc = din("bfc", (128, 10), f32)
    embsel = din("embsel", (1024, NT, BP), bf)

    p1_out = nc.dram_tensor("p1_out", [NT, VP, BP], f32, kind="ExternalOutput")
    p_out = nc.dram_tensor("p_out", [NT, VP, BP], f32, kind="ExternalOutput")

    RG = [list(range(NC))]

    def ag(src_ap, dst_ap):
        nc.gpsimd.collective_compute(
            "AllGather", OP.bypass, replica_groups=RG,
            ins=[src_ap.opt()], outs=[dst_ap.opt()],
        )

    with tile.TileContext(nc) as tc:
      with (
          tc.tile_pool(name="keep", bufs=1) as keep,
          tc.tile_pool(name="kdram", bufs=1, space="DRAM") as kdram,
      ):
        # ---- long-lived SBUF state ----
        att1_sb = keep.tile([128, 4, BP, R], bf, name="att1_sb")
        topicsT_sb = keep.tile([128, 4, BP, NT], bf, name="topicsT_sb")
        imgF_sb = keep.tile([128, 4, BP, R], bf, name="imgF_sb")
        g1c_sb = keep.tile([128, 4, NT, BP], f32, name="g1c_sb")
        h1T_sb = keep.tile([128, 8, BP], bf, name="h1T_sb")
        h2T_sb = keep.tile([128, 8, BP], bf, name="h2T_sb")
        c1_sb = keep.tile([128, BP], f32, name="c1_sb")
        c2_sb = keep.tile([128, BP], f32, name="c2_sb")
        b1_sb = keep.tile([128, 4], f32, name="b1_sb")
        b2_sb = keep.tile([128, 4], f32, name="b2_sb")
        battn_sb = keep.tile([128, 4], f32, name="battn_sb")
        bfc1_sb = keep.tile([128, 10], f32, name="bfc1_sb")
        bfc_sb = keep.tile([128, 10], f32, name="bfc_sb")
        wf_sb = keep.tile([128, 4], bf, name="wf_sb")
        ones_sb = keep.tile([1, 128], bf, name="ones_sb")

        nc.sync.dma_start(b1_sb[:], b1[:])
        nc.sync.dma_start(b2_sb[:], b2[:])
        nc.sync.dma_start(battn_sb[:], battn[:])
        nc.sync.dma_start(bfc1_sb[:], bfc1[:])
        nc.sync.dma_start(bfc_sb[:], bfc[:])
        nc.sync.dma_start(wf_sb[:], wfull[:])
        nc.sync.dma_start(
            imgF_sb[:], img_f[:].rearrange("(ft p) b r -> p ft b r", p=128)
        )
        nc.gpsimd.memset(ones_sb[:], 1.0)
        nc.gpsimd.memset(h1T_sb[:], 0.0)
        nc.gpsimd.memset(h2T_sb[:], 0.0)
        nc.gpsimd.memset(c1_sb[:], 0.0)
        nc.gpsimd.memset(c2_sb[:], 0.0)
        nc.gpsimd.memset(topicsT_sb[:], 0.0)

        imgmean_sh = kdram.tile([F, BS], bf, name="imgmean_sh")
        imgmean_all = kdram.tile([NC, F, BS], bf, name="imgmean_all", addr_space="Shared")
        att1_sh = kdram.tile([A, BS * R], bf, name="att1_sh")
        att1_all = kdram.tile([NC, A, BS * R], bf, name="att1_all", addr_space="Shared")
        lin_poly = kdram.tile([2, BS * R, 512], bf, name="lin_poly")
        top_sh = kdram.tile([BS * 500, 6], bf, name="top_sh")
        top_all = kdram.tile([NC, BS * 500, 6], bf, name="top_all", addr_space="Shared")

        # ============ PREAMBLE ============
        with tc.tile_pool(name="pre_img", bufs=1) as pre_img:
            img_dp_sb = pre_img.tile([128, 32, BS * R], bf, name="img_dp_sb")
            nc.sync.dma_start(
                img_dp_sb[:], img_dp[:].rearrange("(kt p) b r -> p kt (b r)", p=128)
            )
            with tc.tile_pool(name="pre_mean", bufs=1) as pre_mean:
                imgmean_sb = pre_mean.tile([128, 32, BP], bf, name="imgmean_sb")

                # ---- phase A: img_mean + att1 (DP over own 13 samples) ----
                with (
                    tc.tile_pool(name="preA", bufs=1) as preA,
                    tc.tile_pool(name="preA_ps", bufs=2, space="PSUM") as preA_ps,
                ):
                    wfeat_sb = preA.tile([128, 32, A], bf, name="wfeat_sb")
                    nc.sync.dma_start(
                        wfeat_sb[:], w_feat[:].rearrange("(kt p) a -> p kt a", p=128)
                    )
                    imgmean_dp = preA.tile([128, 32, BS], bf, name="imgmean_dp")
                    for kt in range(32):
                        msum = preA.tile(
                            [128, BS], f32, name="msum", tag="msum", bufs=3
                        )
                        nc.vector.tensor_reduce(
                            msum[:],
                            img_dp_sb[:, kt, :].rearrange("p (b r) -> p b r", b=BS),
                            axis=X, op=OP.add,
                        )
                        nc.scalar.activation(
                            imgmean_dp[:, kt, :], msum[:], AF.Copy, scale=1.0 / 15.0
                        )
                    nc.sync.dma_start(
                        imgmean_sh[:].rearrange("(kt p) b -> p kt b", p=128),
                        imgmean_dp[:],
                    )
                    ag(imgmean_sh[:], imgmean_all[:])

                    att1_dp = preA.tile([128, 4, BS * R], bf, name="att1_dp")
                    for at in range(4):
                        ps = preA_ps.tile(
                            [128, BS * R], f32, name="psA", tag="psA"
                        )
                        for kt in range(32):
                            nc.tensor.matmul(
                                ps[:],
                                wfeat_sb[:, kt, 128 * at : 128 * (at + 1)],
                                img_dp_sb[:, kt, :],
                                start=(kt == 0), stop=(kt == 31),
                            )
                        nc.scalar.copy(att1_dp[:, at, :], ps[:])
                    nc.sync.dma_start(
                        att1_sh[:].rearrange("(at p) x -> p at x", p=128), att1_dp[:]
                    )
                    ag(att1_sh[:], att1_all[:])

                    # reload gathered results (per-rank DMAs keep APs <= 4 dims)
                    for rk in range(NC):
                        nc.sync.dma_start(
                            att1_sb[:, :, BS * rk : BS * (rk + 1), :],
                            att1_all[rk].rearrange(
                                "(at p) (b r) -> p at b r", p=128, b=BS
                            ),
                        )
                        nc.sync.dma_start(
                            imgmean_sb[:, :, BS * rk : BS * (rk + 1)],
                            imgmean_all[rk].rearrange("(kt p) b -> p kt b", p=128),
                        )

                # ---- phase B: autoencoder lin + conv topics (DP) ----
                with (
                    tc.tile_pool(name="preB", bufs=1) as preB,
                    tc.tile_pool(name="preB_ps", bufs=2, space="PSUM") as preB_ps,
                ):
                    wlt_sb = preB.tile([128, 32, 1024], bf, name="wlt_sb")
                    nc.sync.dma_start(
                        wlt_sb[:], w_lt[:].rearrange("(kt p) m -> p kt m", p=128)
                    )
                    blt_sb = preB.tile([128, 1024], f32, name="blt_sb")
                    nc.sync.dma_start(blt_sb[:], blt[:])
                    cw_sb = preB.tile([120, 4, 6], bf, name="cw_sb")
                    nc.sync.dma_start(cw_sb[:], cwt[:])
                    cbt_sb = preB.tile([128, 6], f32, name="cbt_sb")
                    nc.sync.dma_start(cbt_sb[:], cbt[:])

                    # lin batch-major: (b*R rows, 1024) via lhsT=img tiles
                    lin_bm = preB.tile([128, 2, 1024], bf, name="lin_bm")
                    for Mt in range(2):
                        pn = 128 if Mt == 0 else BS * R - 128
                        for ch in range(2):
                            ps = preB_ps.tile(
                                [128, 512], f32, name="psB", tag="psB"
                            )
                            for kt in range(32):
                                nc.tensor.matmul(
                                    ps[:pn, :],
                                    img_dp_sb[:, kt, 128 * Mt : 128 * Mt + pn],
                                    wlt_sb[:, kt, 512 * ch : 512 * (ch + 1)],
                                    start=(kt == 0), stop=(kt == 31),
                                )
                            nc.vector.tensor_tensor(
                                lin_bm[:pn, Mt, 512 * ch : 512 * (ch + 1)],
                                ps[:pn, :],
                                blt_sb[:pn, 512 * ch : 512 * (ch + 1)],
                                op=OP.add,
                            )
                    # polyphase split on DVE (free stride-2), then to DRAM
                    lin_pq = preB.tile([128, 2, 2, 512], bf, name="lin_pq")
                    for Mt in range(2):
                        pn = 128 if Mt == 0 else BS * R - 128
                        for q in range(2):
                            nc.vector.tensor_copy(
                                lin_pq[:pn, Mt, q, :], lin_bm[:pn, Mt, q::2]
                            )
                            nc.sync.dma_start(
                                lin_poly[q, 128 * Mt : 128 * Mt + pn, :],
                                lin_pq[:pn, Mt, q, :],
                            )

                    # im2col + conv matmuls, w in halves of 250
                    lin_poly_h = lin_poly[:].tensor
                    top_h = top_sh[:].tensor
                    for h in range(2):
                        imcol = preB.tile(
                            [120, 4, BS, 250], bf, name="imcol", tag="imcol"
                        )
                        nc.gpsimd.memset(imcol[:, 3, :, :], 0.0)
                        for pair in range(26):
                            q, v = pair // 13, pair % 13
                            kt, g = pair // 8, pair % 8
                            sap = AP(
                                lin_poly_h,
                                q * (BS * R * 512) + v + 250 * h,
                                [[512, 15], [R * 512, BS], [1, 250]],
                            )
                            nc.sync.dma_start(
                                imcol[15 * g : 15 * g + 15, kt, :, :], sap
                            )
                        tbw = preB.tile(
                            [125, BS, 2, 6], bf, name="tbw", tag="tbw", bufs=2
                        )
                        for b in range(BS):
                            for qq in range(2):
                                psc = preB_ps.tile(
                                    [125, 6], f32, name="psC", tag="psC", bufs=2
                                )
                                for kt in range(4):
                                    nc.tensor.matmul(
                                        psc[:],
                                        imcol[:, kt, b, 125 * qq : 125 * (qq + 1)],
                                        cw_sb[:, kt, :],
                                        start=(kt == 0), stop=(kt == 3),
                                    )
                                nc.vector.tensor_tensor(
                                    tbw[:, b, qq, :], psc[:], cbt_sb[:125, :],
                                    op=OP.add,
                                )
                        for qq in range(2):
                            dst = AP(
                                top_h,
                                (250 * h + 125 * qq) * 6,
                                [[6, 125], [500 * 6, BS], [1, 6]],
                            )
                            nc.sync.dma_start(dst, tbw[:, :, qq, :])
                    ag(top_sh[:], top_all[:])
                    # reload: topicsT_sb[p, wt, b, c]
                    topall_h = top_all[:].tensor
                    for wt in range(4):
                        pn = 128 if wt < 3 else 116
                        for rk in range(NC):
                            sap = AP(
                                topall_h,
                                rk * BS * 500 * 6 + 128 * wt * 6,
                                [[6, pn], [500 * 6, BS], [1, 6]],
                            )
                            nc.sync.dma_start(
                                topicsT_sb[
                                    :pn, wt, BS * rk : BS * (rk + 1), :NT
                                ].rearrange("p b c -> p b c")
                                if False
                                else topicsT_sb[:pn, wt, BS * rk : BS * (rk + 1), :],
                                AP(
                                    topall_h,
                                    rk * BS * 500 * 6 + 128 * wt * 6,
                                    [[6, pn], [500 * 6, BS], [1, 5]],
                                ),
                            )


